# revision 49
# baseline (speedup 1.0000x reference)
"""Trainium2 Bass kernel for MllamaTextSdpaAttention (GQA + RoPE + causal SDPA).

Strategy: tensor-parallel over heads across 8 NeuronCores. Core c owns
q-heads [4c, 4c+4) and kv-head c (kv groups intact). Each core computes
hidden @ Wq/Wk/Wv slices, RoPE, causal attention for its heads, and its
row-slice of the Wo matmul, yielding a partial [T, DIM] output (bf16).
The host sums the 8 partials in f32.

Speed tricks on top of the bf16 layout (prior version):
- All four projection GEMMs run as fp8e4m3 DoubleRow matmuls. Each
  operand X is split host-side (or on-chip for the attention output)
  into an fp8 pair (X_hi, X_lo) at a shared power-of-2 scale with
  X ~= X_hi + X_lo, and the product uses the three terms
  hi*hi + lo*hi + hi*lo (lo*lo dropped, ~1e-3 rel). DoubleRow
  contracts two 128-deep k-tiles per instruction at 0.5 cycles/column,
  so the 3-term product costs 0.75x the bf16 schedule while matching
  bf16 accuracy. Scores, softmax rowsum, and P@V stay bf16: fp8
  cannot span exp(score)'s dynamic range (peaked rows lose the
  denominator; diffuse rows flush to zero), and single-fp8 score
  operands breach the 2e-2 error budget through the softmax's
  absolute-error sensitivity.
- Weights arrive in partition-major DRAM layouts (>=512B contiguous
  per partition) to dodge the 2x small-element DMA penalty; output
  writes are paired to beat the per-DMA descriptor floor; all
  constant descales fold into the rowsum ones-vector / rope tables /
  one host-side multiply, so every PSUM->SBUF copy is a plain copy.
- The prologue DMA stream is need-ordered and the PE is kept ramped
  through the DMA-paced startup with cheap filler matmuls (the
  tensor engine drops to a lower p-state after idling, which would
  otherwise double the cost of the first ~15us of real matmuls).
- Elementwise work is spread across ACT/DVE/Pool (attention-output
  fp8 split on Pool+DVE, copies alternating ACT/DVE).

Layout details carried over from the bf16 version:
- hidden_states fed transposed ([DIM, T]) so projections contract on
  partitions; Q^T/K^T produced directly (head_dim on partitions).
- RoPE de-interleaved by permuting Wq/Wk columns host-side -> plain
  half-rotation with partition-offset slices; 1/sqrt(d) and all fp8
  descales folded into the cos/sin tables / copy scales.
- Scores computed transposed: scT[k, q] = K_rot^T(tile).T @ Q_rot^T, so
  exp(scT) feeds P@V directly (out^T = V.T @ expT) with no transposes.
- Causality at 128-block granularity; diagonal blocks get the
  transposed additive mask; no max-subtraction (scores bounded).
- 1/rowsum broadcast on GpSimd; each group's epilogue (which now also
  produces the fp8 hi/lo pair of the attention output) deferred into
  the next group so the PE never waits on the DVE.
- Emission interleaves projection chunks with the attention groups they
  unblock; all [128,512]-f32 PSUM scratch shares one 5-slot pool
  (+2 ot +1 rs = 8 banks).
"""

import numpy as np
import ml_dtypes

import concourse.bacc as bacc
import concourse.bass as bass
import concourse.mybir as mybir
from concourse.tile import TileContext
from concourse import bass_utils

BF16 = mybir.dt.bfloat16
F32 = mybir.dt.float32
F8 = mybir.dt.float8e4
F8E5 = mybir.dt.float8e5
NPF8 = ml_dtypes.float8_e4m3

B, S, DIM = 2, 1024, 4096
T = B * S                     # 2048 tokens, batch-major
N_HEADS, N_KV = 32, 8
HD = 128                      # head dim == partition count
N_CORES = 8
HL = N_HEADS // N_CORES       # 4 local q-heads per core
KT = DIM // 128               # 32 feature tiles
CH = 512                      # projection token-chunk
NCHUNK = T // CH
QB = 512                      # attention q-block width
TT = T // 128                 # 16 token tiles global
SCALE = 1.0 / float(np.sqrt(HD))

S_H = 8.0                     # hidden-states fp8 scale
S_W = 64.0                    # weights fp8 scale
S_A = 32.0                    # attention-output fp8 scale
PROJ_DESCALE = 1.0 / (S_H * S_W)
# v_sb holds (S_H*S_W)*v_true; the rowsum's ones-vector value folds every
# remaining descale so tmp = ot * (1/rowsum_psum) lands at S_A * attn_out:
#   rowsum_psum = ONES_VAL * rowsum_true
#   tmp = (S_H*S_W*ot_true) / (ONES_VAL * rowsum_true) == S_A * attn
ONES_VAL = S_H * S_W / S_A    # 16, exact in bf16
# final out-proj psum = (S_A*attn) @ (S_W*wo) = HOST_DESCALE^-1 * out
HOST_DESCALE = 1.0 / (S_A * S_W)

DR = mybir.MatmulPerfMode.DoubleRow

_CACHE: dict = {}


def _build():
    nc = bacc.Bacc("TRN2", target_bir_lowering=False, debug=False,
                   enable_asserts=False)

    # weights arrive pre-transposed to partition-major layouts so every DMA
    # moves >=512B contiguous runs per partition (avoids the small-element
    # DMA penalty)
    hs_hi = nc.dram_tensor("hs_hi", [DIM, T], F8, kind="ExternalInput")
    hs_lo = nc.dram_tensor("hs_lo", [DIM, T], F8, kind="ExternalInput")
    wq_hi = nc.dram_tensor("wq_hi", [128, HL, KT, HD], F8, kind="ExternalInput")
    wq_lo = nc.dram_tensor("wq_lo", [128, HL, KT, HD], F8, kind="ExternalInput")
    wk_hi = nc.dram_tensor("wk_hi", [128, KT, HD], F8, kind="ExternalInput")
    wk_lo = nc.dram_tensor("wk_lo", [128, KT, HD], F8, kind="ExternalInput")
    wv_hi = nc.dram_tensor("wv_hi", [128, KT, HD], F8, kind="ExternalInput")
    wv_lo = nc.dram_tensor("wv_lo", [128, KT, HD], F8, kind="ExternalInput")
    wo_hi = nc.dram_tensor("wo_hi", [128, HL, DIM], F8, kind="ExternalInput")
    wo_lo = nc.dram_tensor("wo_lo", [128, HL, DIM], F8, kind="ExternalInput")
    cos_q = nc.dram_tensor("cos_q", [HD, T], BF16, kind="ExternalInput")
    sin_q = nc.dram_tensor("sin_q", [HD, T], BF16, kind="ExternalInput")
    cos_k = nc.dram_tensor("cos_k", [HD, T], BF16, kind="ExternalInput")
    sin_k = nc.dram_tensor("sin_k", [HD, T], BF16, kind="ExternalInput")
    maskT = nc.dram_tensor("maskT", [128, 128], F32, kind="ExternalInput")
    out = nc.dram_tensor("out", [T, DIM], BF16, kind="ExternalOutput")

    Exp = mybir.ActivationFunctionType.Exp

    with TileContext(nc) as tc:
        with tc.tile_pool(name="consts", bufs=1) as cpool, \
             tc.tile_pool(name="hs", bufs=2) as hpool, \
             tc.tile_pool(name="rope_tmp", bufs=1) as rpool, \
             tc.tile_pool(name="work_ps", bufs=5, space=bass.MemorySpace.PSUM) as wpool, \
             tc.tile_pool(name="ot_ps", bufs=2, space=bass.MemorySpace.PSUM) as otpool, \
             tc.tile_pool(name="rs_ps", bufs=1, space=bass.MemorySpace.PSUM) as rspool, \
             tc.tile_pool(name="et", bufs=5) as epool, \
             tc.tile_pool(name="bc_sb", bufs=2) as bcsbpool, \
             tc.tile_pool(name="recip", bufs=2) as rcpool, \
             tc.tile_pool(name="ao_tmp", bufs=2) as aopool, \
             tc.tile_pool(name="out_sb", bufs=4) as xsbpool:

            wqh = [cpool.tile([128, KT, HD], F8, tag=f"wqh{m}", name=f"wqh{m}")
                   for m in range(HL)]
            wql = [cpool.tile([128, KT, HD], F8, tag=f"wql{m}", name=f"wql{m}")
                   for m in range(HL)]
            wkh_sb = cpool.tile([128, KT, HD], F8, tag="wkh")
            wkl_sb = cpool.tile([128, KT, HD], F8, tag="wkl")
            wvh_sb = cpool.tile([128, KT, HD], F8, tag="wvh")
            wvl_sb = cpool.tile([128, KT, HD], F8, tag="wvl")
            cq_sb = cpool.tile([128, T], BF16, tag="cq")
            sq_sb = cpool.tile([128, T], BF16, tag="sq")
            ck_sb = cpool.tile([128, T], BF16, tag="ck")
            sk_sb = cpool.tile([128, T], BF16, tag="sk")
            maskT_sb = cpool.tile([128, 128], F32, tag="maskT")
            ones_k = cpool.tile([128, 1], BF16, tag="ones_k")
            warm_sb = cpool.tile([128, 512], BF16, tag="warm")
            qt_rot = cpool.tile([128, HL, T], BF16, tag="qt")
            kt_rot = cpool.tile([128, T], BF16, tag="kt")
            v_sb = cpool.tile([128, TT, HD], BF16, tag="v")
            ao_hi = cpool.tile([128, HL, T], F8, tag="aoh")
            ao_lo = cpool.tile([128, HL, T], F8, tag="aol")

            hs_hi_r = hs_hi.ap().rearrange("(kt p) t -> p kt t", p=128)
            hs_lo_r = hs_lo.ap().rearrange("(kt p) t -> p kt t", p=128)

            # startup-critical DMA first: what the first Q chain touches.
            # The rest of head 0's weights interleave with chunk 0's hs
            # slices (emit_chunk's dma_hook) in need-order.
            nc.sync.dma_start(wqh[0][:, 0:4, :], wq_hi.ap()[:, 0, 0:4, :])
            nc.vector.memset(warm_sb[:, 0:8], 1.0)
            nc.vector.memset(warm_sb[:, 8:512], 1.0)

            def warm(n):
                """Keep the PE p-state ramped through DMA-paced stalls:
                dummy matmuls (213ns each) into the rs-pool scratch bank."""
                for _ in range(n):
                    wps = rspool.tile([1, QB], F32, tag="rs", name="warm_ps")
                    nc.tensor.matmul(wps, warm_sb[:, 0:1], warm_sb,
                                     start=True, stop=True)

            warm(14)

            def chunk0_weight_dmas(g, lo):
                if not lo:
                    if g == 0:
                        nc.sync.dma_start(wqh[0][:, 4:KT, :],
                                          wq_hi.ap()[:, 0, 4:KT, :])
                    elif g == 2:
                        nc.sync.dma_start(wql[0], wq_lo.ap()[:, 0, :, :])
                elif g == 1:
                    nc.sync.dma_start(cq_sb, cos_q.ap())
                    nc.sync.dma_start(sq_sb, sin_q.ap())

            def late_consts():
                for m in range(1, HL):
                    nc.sync.dma_start(wqh[m], wq_hi.ap()[:, m, :, :])
                    nc.sync.dma_start(wql[m], wq_lo.ap()[:, m, :, :])
                nc.sync.dma_start(wkh_sb, wk_hi.ap())
                nc.sync.dma_start(wkl_sb, wk_lo.ap())
                nc.sync.dma_start(ck_sb, cos_k.ap())
                nc.sync.dma_start(sk_sb, sin_k.ap())
                nc.sync.dma_start(wvh_sb, wv_hi.ap())
                nc.sync.dma_start(wvl_sb, wv_lo.ap())
                nc.sync.dma_start(maskT_sb, maskT.ap())
                nc.vector.memset(ones_k, ONES_VAL)

            def rope(ps, out_ap, cos_ap, sin_ap):
                """out = ps*cos + halfswap(ps)*sin  (signs baked into sin)."""
                t1 = rpool.tile([128, CH], F32, tag="r1", name="t1")
                t2 = rpool.tile([128, CH], F32, tag="r2", name="t2")
                nc.vector.tensor_mul(t1, ps, cos_ap)
                nc.vector.tensor_mul(t2[0:64, :], ps[64:128, :], sin_ap[0:64, :])
                nc.vector.tensor_mul(t2[64:128, :], ps[0:64, :], sin_ap[64:128, :])
                nc.vector.tensor_add(out_ap, t1, t2)

            def proj_3term(ps, w_hi, w_lo, m_hi, m_lo, hook=None,
                           warm_fill=False):
                """48 DoubleRow matmuls: w_hi*m_hi + w_lo*m_hi + w_hi*m_lo."""
                NP = KT // 2
                for j in range(NP):
                    nc.tensor.matmul(ps, w_hi[:, 2 * j:2 * j + 2, :],
                                     m_hi[:, 2 * j:2 * j + 2, :],
                                     start=(j == 0), stop=False, perf_mode=DR)
                    if warm_fill and j % 4 == 3 and j < NP - 1:
                        warm(6)
                if hook is not None:
                    hook()
                for j in range(NP):
                    nc.tensor.matmul(ps, w_lo[:, 2 * j:2 * j + 2, :],
                                     m_hi[:, 2 * j:2 * j + 2, :],
                                     start=False, stop=False, perf_mode=DR)
                for j in range(NP):
                    nc.tensor.matmul(ps, w_hi[:, 2 * j:2 * j + 2, :],
                                     m_lo[:, 2 * j:2 * j + 2, :],
                                     start=False, stop=(j == NP - 1),
                                     perf_mode=DR)
                    if warm_fill and j % 4 == 3 and j < NP - 1:
                        warm(6)

            def emit_chunk(c, dma_hook=None):
                t0 = c * CH
                hsh = hpool.tile([128, KT, CH], F8, tag="hsh", name="hsh")
                hsl = hpool.tile([128, KT, CH], F8, tag="hsl", name="hsl")
                nslc = 4
                kper = KT // nslc
                for g in range(nslc):
                    nc.sync.dma_start(
                        hsh[:, g * kper:(g + 1) * kper, :],
                        hs_hi_r[:, g * kper:(g + 1) * kper, t0:t0 + CH])
                    if dma_hook is not None:
                        dma_hook(g, False)
                for g in range(nslc):
                    nc.sync.dma_start(
                        hsl[:, g * kper:(g + 1) * kper, :],
                        hs_lo_r[:, g * kper:(g + 1) * kper, t0:t0 + CH])
                    if dma_hook is not None:
                        dma_hook(g, True)
                for m in range(HL):
                    ps = wpool.tile([128, CH], F32, tag="work", name="ps_q")
                    proj_3term(ps, wqh[m], wql[m], hsh, hsl,
                               hook=late_consts if (c == 0 and m == 0) else None,
                               warm_fill=(c == 0 and m == 0))
                    rope(ps, qt_rot[:, m, t0:t0 + CH],
                         cq_sb[:, t0:t0 + CH], sq_sb[:, t0:t0 + CH])
                ps = wpool.tile([128, CH], F32, tag="work", name="ps_k")
                proj_3term(ps, wkh_sb, wkl_sb, hsh, hsl)
                rope(ps, kt_rot[:, t0:t0 + CH],
                     ck_sb[:, t0:t0 + CH], sk_sb[:, t0:t0 + CH])
                for vi in range(CH // 128):
                    tt = t0 // 128 + vi
                    ps = wpool.tile([128, HD], F32, tag="work", name="ps_v")
                    sl = slice(vi * 128, (vi + 1) * 128)
                    NP = KT // 2
                    for j in range(NP):
                        nc.tensor.matmul(ps, hsh[:, 2 * j:2 * j + 2, sl],
                                         wvh_sb[:, 2 * j:2 * j + 2, :],
                                         start=(j == 0), stop=False,
                                         perf_mode=DR)
                    for j in range(NP):
                        nc.tensor.matmul(ps, hsl[:, 2 * j:2 * j + 2, sl],
                                         wvh_sb[:, 2 * j:2 * j + 2, :],
                                         start=False, stop=False, perf_mode=DR)
                    for j in range(NP):
                        nc.tensor.matmul(ps, hsh[:, 2 * j:2 * j + 2, sl],
                                         wvl_sb[:, 2 * j:2 * j + 2, :],
                                         start=False, stop=(j == NP - 1),
                                         perf_mode=DR)
                    if vi % 2 == 0:
                        nc.scalar.copy(v_sb[:, tt, :], ps)
                    else:
                        nc.vector.tensor_copy(v_sb[:, tt, :], ps)

            # --- attention group machinery (transposed-scores scheme) ---
            pending = [None]

            def epilogue(st):
                rs, ot, h, q0 = st
                recip = rcpool.tile([1, QB], F32, tag="recip", name="recip")
                nc.vector.reciprocal(recip, rs)
                bcs = bcsbpool.tile([128, QB], F32, tag="bcs", name="bcs")
                nc.gpsimd.partition_broadcast(bcs, recip)
                tmp = aopool.tile([128, QB], F32, tag="aot", name="tmp")
                nc.vector.tensor_mul(tmp, ot, bcs)
                nc.gpsimd.tensor_copy(ao_hi[:, h, q0:q0 + QB], tmp)
                nc.vector.tensor_sub(ao_lo[:, h, q0:q0 + QB], tmp,
                                     ao_hi[:, h, q0:q0 + QB])

            def emit_group(b, h, qb):
                q0 = b * S + qb * QB
                n_kt = (qb + 1) * (QB // 128)
                rs = rspool.tile([1, QB], F32, tag="rs", name="rs")
                ot = otpool.tile([128, QB], F32, tag="ot", name="ot")
                ets = [None] * n_kt

                def c0_of(kt):
                    return max(0, kt - qb * (QB // 128)) * 128

                def emit_sc(kt):
                    c0 = c0_of(kt)
                    sc = wpool.tile([128, QB], F32, tag="work", name="sc")
                    nc.tensor.matmul(
                        sc[:, c0:],
                        kt_rot[:, b * S + kt * 128:b * S + (kt + 1) * 128],
                        qt_rot[:, h, q0 + c0:q0 + QB],
                        start=True, stop=True)
                    jd = kt - qb * (QB // 128)
                    if 0 <= jd < QB // 128:
                        nc.vector.tensor_add(sc[:, jd * 128:(jd + 1) * 128],
                                             sc[:, jd * 128:(jd + 1) * 128],
                                             maskT_sb)
                    et = epool.tile([128, QB], BF16, tag="et", name="et")
                    nc.scalar.activation(et[:, c0:], sc[:, c0:], Exp,
                                         bias=0.0, scale=1.0)
                    ets[kt] = (et, c0)

                for w in range(min(4, n_kt)):
                    emit_sc(w)
                for kt in range(n_kt):
                    if kt + 4 < n_kt:
                        emit_sc(kt + 4)
                    et, c0 = ets[kt]
                    nc.tensor.matmul(rs[:, c0:], ones_k, et[:, c0:],
                                     start=(kt == 0), stop=(kt == n_kt - 1))
                    nc.tensor.matmul(ot[:, c0:], v_sb[:, b * (S // 128) + kt, :],
                                     et[:, c0:], start=(kt == 0),
                                     stop=(kt == n_kt - 1))
                    ets[kt] = None
                    if kt == 0 and pending[0] is not None:
                        epilogue(pending[0])
                        pending[0] = None
                pending[0] = (rs, ot, h, q0)

            # --- interleaved emission: each chunk unblocks a set of groups ---
            # chunk c covers tokens [c*512, (c+1)*512) = batch c//2, q-block c%2
            woh_sb = None
            wol_sb = None
            for c in range(NCHUNK):
                emit_chunk(c, dma_hook=chunk0_weight_dmas if c == 0 else None)
                b, qb = c // 2, c % 2
                for h in range(HL):
                    emit_group(b, h, qb)
            # wo reuses hs slots (same size); loaded after the last chunk's
            # groups (in pieces) so it never contends with hs traffic, and
            # overlaps the tail attention work
            woh_sb = hpool.tile([128, HL, DIM], F8, tag="hsh", name="woh_sb")
            wol_sb = hpool.tile([128, HL, DIM], F8, tag="hsl", name="wol_sb")
            for m in range(HL):
                nc.sync.dma_start(woh_sb[:, m, :], wo_hi.ap()[:, m, :])
            for m in range(HL):
                nc.sync.dma_start(wol_sb[:, m, :], wo_lo.ap()[:, m, :])
            if pending[0] is not None:
                epilogue(pending[0])
                pending[0] = None

            # ---- output projection (row-parallel Wo, 3-term fp8 DR) ----
            for tt in range(TT):
                ts = slice(tt * 128, (tt + 1) * 128)
                for nj in range(DIM // 1024):
                    osb = xsbpool.tile([128, 2, 512], BF16, tag="osb",
                                       name="osb")
                    for half in range(2):
                        n0 = nj * 1024 + half * 512
                        ns = slice(n0, n0 + 512)
                        ps = wpool.tile([128, 512], F32, tag="work",
                                        name="ps_o")
                        for m2 in range(HL // 2):
                            hs2 = slice(2 * m2, 2 * m2 + 2)
                            nc.tensor.matmul(ps, ao_hi[:, hs2, ts],
                                             woh_sb[:, hs2, ns],
                                             start=(m2 == 0), stop=False,
                                             perf_mode=DR)
                        for m2 in range(HL // 2):
                            hs2 = slice(2 * m2, 2 * m2 + 2)
                            nc.tensor.matmul(ps, ao_lo[:, hs2, ts],
                                             woh_sb[:, hs2, ns],
                                             start=False, stop=False,
                                             perf_mode=DR)
                        for m2 in range(HL // 2):
                            hs2 = slice(2 * m2, 2 * m2 + 2)
                            nc.tensor.matmul(ps, ao_hi[:, hs2, ts],
                                             wol_sb[:, hs2, ns],
                                             start=False,
                                             stop=(m2 == HL // 2 - 1),
                                             perf_mode=DR)
                        if (nj * 2 + half) % 2 == 0:
                            nc.scalar.copy(osb[:, half, :], ps)
                        else:
                            nc.vector.tensor_copy(osb[:, half, :], ps)
                        if tt == TT - 1 and nj == DIM // 1024 - 1:
                            # split the final writes so the drain tail is
                            # as short as possible
                            nc.sync.dma_start(
                                out.ap()[ts, nj * 1024 + half * 512:
                                         nj * 1024 + half * 512 + 512],
                                osb[:, half, :])
                    if not (tt == TT - 1 and nj == DIM // 1024 - 1):
                        nc.sync.dma_start(
                            out.ap()[ts, nj * 1024:(nj + 1) * 1024],
                            osb.rearrange("p a b -> p (a b)"))
    nc.compile()
    return nc


def _get_nc():
    if "nc" not in _CACHE:
        _CACHE["nc"] = _build()
    return _CACHE["nc"]


def _split_f8(x):
    hi = x.astype(NPF8)
    lo = (x - hi.astype(np.float32)).astype(NPF8)
    return hi, lo


def _prep_inputs(inputs) -> list[dict]:
    bf16 = ml_dtypes.bfloat16
    hs = np.asarray(inputs["hidden_states"], dtype=np.float32).reshape(T, DIM)
    hsT = np.ascontiguousarray(hs.T) * S_H
    hs_hi, hs_lo = _split_f8(hsT)

    fc = np.asarray(inputs["freqs_cos"], dtype=np.float32).reshape(T, HD // 2).T
    fs = np.asarray(inputs["freqs_sin"], dtype=np.float32).reshape(T, HD // 2).T
    cos2 = np.concatenate([fc, fc], axis=0)            # [128, T]
    sin2 = np.concatenate([-fs, fs], axis=0)           # signed half-rotation
    cos_qv = np.ascontiguousarray(cos2 * (SCALE * PROJ_DESCALE)).astype(bf16)
    sin_qv = np.ascontiguousarray(sin2 * (SCALE * PROJ_DESCALE)).astype(bf16)
    cos_kv = np.ascontiguousarray(cos2 * PROJ_DESCALE).astype(bf16)
    sin_kv = np.ascontiguousarray(sin2 * PROJ_DESCALE).astype(bf16)

    maskT = np.ascontiguousarray(
        np.asarray(inputs["attention_mask"], dtype=np.float32)[0, 0, :128, :128].T)

    perm = np.concatenate([np.arange(0, HD, 2), np.arange(1, HD, 2)])
    Wq = np.asarray(inputs["Wq"], dtype=np.float32) * S_W
    Wk = np.asarray(inputs["Wk"], dtype=np.float32) * S_W
    Wv = np.asarray(inputs["Wv"], dtype=np.float32) * S_W
    Wo = np.asarray(inputs["Wo"], dtype=np.float32) * S_W

    in_maps = []
    for c in range(N_CORES):
        wq_c = np.concatenate(
            [Wq[:, (c * HL + h) * HD:(c * HL + h + 1) * HD][:, perm]
             for h in range(HL)], axis=1)
        wk_c = Wk[:, c * HD:(c + 1) * HD][:, perm]
        wv_c = Wv[:, c * HD:(c + 1) * HD]
        wo_c = Wo[c * HL * HD:(c + 1) * HL * HD, :]
        # partition-major layouts (see dram_tensor comments in _build)
        wq_c = wq_c.reshape(KT, 128, HL, HD).transpose(1, 2, 0, 3)
        wk_c = wk_c.reshape(KT, 128, HD).transpose(1, 0, 2)
        wv_c = wv_c.reshape(KT, 128, HD).transpose(1, 0, 2)
        wo_c = wo_c.reshape(HL, 128, DIM).transpose(1, 0, 2)
        wq_h, wq_l = _split_f8(np.ascontiguousarray(wq_c))
        wk_h, wk_l = _split_f8(np.ascontiguousarray(wk_c))
        wv_h, wv_l = _split_f8(np.ascontiguousarray(wv_c))
        wo_h, wo_l = _split_f8(np.ascontiguousarray(wo_c))
        in_maps.append({
            "hs_hi": hs_hi, "hs_lo": hs_lo,
            "wq_hi": wq_h, "wq_lo": wq_l,
            "wk_hi": wk_h, "wk_lo": wk_l,
            "wv_hi": wv_h, "wv_lo": wv_l,
            "wo_hi": wo_h, "wo_lo": wo_l,
            "cos_q": cos_qv, "sin_q": sin_qv,
            "cos_k": cos_kv, "sin_k": sin_kv,
            "maskT": maskT,
        })
    return in_maps


def kernel(**inputs) -> np.ndarray:
    nc = _get_nc()
    in_maps = _prep_inputs(inputs)
    res = bass_utils.run_bass_kernel_spmd(nc, in_maps,
                                          core_ids=list(range(N_CORES)))
    acc = np.zeros((T, DIM), dtype=np.float32)
    for c in range(N_CORES):
        acc += np.asarray(res.results[c]["out"], dtype=np.float32)
    acc *= HOST_DESCALE
    return acc.reshape(B, S, DIM)
